# revision 7
# baseline (speedup 1.0000x reference)
"""Trainium2 Bass kernel for CTRLightGCN-style GNN message passing block.

Reference computation (per full input):
    A_g = row_normalized(A.sum(0)) + A_group                    # (4,25,25)
    xg = x.reshape(B, 4, 64, T, V)
    y  = einsum('gdc,gvw,bgctw->bgdtv', conv_w, A_g, xg).reshape(B, C, T, V)
    out = x + BN_train(y) * gamma + beta        (BN stats over B,T,V per C)

Strategy (v2): data-parallel over batch B=64 across 8 cores (8 per core).
Per core, per (b, channel-half) the two contractions run as a PE matmul
chain (fp16 inputs, fp32 PSUM accum):

  MM1:  lhsT = x16 chunk (stationary) x wblk  -> y1T chunk in PSUM
  MM2:  lhsT = y1T slice x kron(I_5, A_g^T)   -> y chunk (gd x (t,v)) PSUM

v2 changes vs baseline (251us):
 - fp16 DRAM output (host upcasts): halves pass-2 write traffic.
 - x half-1 tiles stay resident in SBUF through pass 1, so pass-2 h1
   needs no x re-read (saves 6.8MB DMA and shortens the exposed tail).
 - per-channel sums fused into the PSUM->SBUF y copy via accum_out
   (Act/DVE alternating); second moment via one DVE tensor_tensor_reduce
   on the fp16 y slab. No bn_stats pass.
 - cross-core stats combine via AllGather (4.6us floor) + 3 local adds
   instead of AllReduce (~21us); a dummy AllGather at t=0 warms the CC
   stream.
 - pass-2 out = (y*gh + dl) + x runs stage-1 on the otherwise idle Pool
   engine (1-input tensor_scalar, SBUF-only) or Act (Identity act), and
   stage-2 + output DMA on DVE; pass-2 h0 is interleaved into pass-1 h1
   so it hides in the AllGather shadow.
"""
import numpy as np

import concourse.bacc as bacc
import concourse.tile as tile
from concourse import mybir
from concourse.bass_utils import run_bass_kernel_spmd

# ---- problem constants (hardcoded per contract) ----
B, C, T, V = 64, 256, 128, 25
G = 4
N_CORES = 8
B_LOC = B // N_CORES          # 8
TW = T * V                    # 3200
TW_PAD = 3328                 # x16 padded so every 128-col lhsT read is in-bounds
BN_EPS = 1e-5
N_PER_CH = B * TW             # 204800 (global per-channel count)

# chunk = 5 t-rows = 125 cols (last chunk 3 t = 75); batches of 4 chunks -> <=500 cols
CHUNK_M = [125] * 25 + [75]
BATCHES = []                  # list of (f0, [m...]) per (b,h)
_f = 0
_i = 0
while _i < len(CHUNK_M):
    ms = CHUNK_M[_i:_i + 4]
    if sum(ms) > 500:
        ms = CHUNK_M[_i:_i + 2]
    BATCHES.append((_f, ms))
    _f += sum(ms)
    _i += len(ms)
N_BAT = len(BATCHES)          # 7 (6x500 + 1x200)
N_REC = B_LOC * N_BAT         # 56 stat records per half

ACT_COPYS = (2, 5)            # batches whose y copy (+sum) runs on Act
ACT_ST1 = (0, 3, 6)           # pass-2 h1 units whose stage-1 runs on Act

F32 = mybir.dt.float32
F16 = mybir.dt.float16
ALU = mybir.AluOpType

_cache = {}


def _build():
    nc = bacc.Bacc()
    x16_in = nc.dram_tensor("x16", [B_LOC, 2, 128, TW_PAD], F16, kind="ExternalInput")
    wblk_in = nc.dram_tensor("wblk", [2, 128, 128], F16, kind="ExternalInput")
    arhs_in = nc.dram_tensor("arhs", [G, 125, 125], F16, kind="ExternalInput")
    gbn_in = nc.dram_tensor("gbn", [2, 128, 2], F32, kind="ExternalInput")
    out_d = nc.dram_tensor("out", [B_LOC, C, TW], F16, kind="ExternalOutput")

    with tile.TileContext(nc) as tc:
        with (
            tc.tile_pool(name="consts", bufs=1) as consts,
            tc.tile_pool(name="resid", bufs=1) as resid,
            tc.tile_pool(name="xres", bufs=1) as xres_p,
            tc.tile_pool(name="xp", bufs=3) as xp,
            tc.tile_pool(name="tmp16", bufs=3) as tmp16p,
            tc.tile_pool(name="y1t", bufs=3) as y1tp,
            tc.tile_pool(name="ps1", bufs=3, space="PSUM") as ps1,
            tc.tile_pool(name="ps2", bufs=4, space="PSUM") as ps2,
            tc.tile_pool(name="psw", bufs=1, space="PSUM") as psw,
            tc.tile_pool(name="dr", bufs=1, space="DRAM") as dr,
        ):
            # ---- PE HAM warmup (~4us of dummy matmuls) ----
            wtile = consts.tile([128, 128], F16, tag="warm")
            nc.vector.memset(wtile, 0.0)
            wp = psw.tile([128, 128], F32, tag="warmp")
            for _ in range(48):
                nc.tensor.matmul(wp, wtile, wtile, start=True, stop=True)
            wsink = consts.tile([128, 1], F32, tag="wsink")
            nc.scalar.copy(out=wsink, in_=wp[:, 0:1])

            # ---- CC-stream warmup: tiny dummy AllGather ----
            zt = consts.tile([128, 2], F32, tag="zt")
            nc.vector.memset(zt, 0.0)
            ccw_in = dr.tile([128, 2], F32, name="ccwi")
            ccw_out = dr.tile([8, 128, 2], F32, addr_space="Shared", name="ccwo")
            nc.gpsimd.dma_start(out=ccw_in, in_=zt)
            nc.gpsimd.collective_compute(
                "AllGather", ALU.bypass,
                replica_groups=[list(range(N_CORES))],
                ins=[ccw_in[:, :]], outs=[ccw_out[:, :, :]],
            )

            # ---- constants ----
            wblk_t = []
            gbn_t = []
            arhs_t = []
            for h in range(2):
                w = consts.tile([128, 128], F16, tag=f"wblk{h}")
                nc.sync.dma_start(out=w, in_=wblk_in[h])
                wblk_t.append(w)
                gbt = consts.tile([128, 2], F32, tag=f"gbn{h}")
                nc.sync.dma_start(out=gbt, in_=gbn_in[h])
                gbn_t.append(gbt)
            for g in range(G):
                a = consts.tile([125, 125], F16, tag=f"arhs{g}")
                nc.sync.dma_start(out=a, in_=arhs_in[g])
                arhs_t.append(a)
            eps_t = consts.tile([128, 1], F32, tag="eps")
            nc.vector.memset(eps_t, BN_EPS)

            y16 = [resid.tile([128, B_LOC, TW], F16, tag=f"y16_{h}", name=f"y16_{h}")
                   for h in range(2)]
            statsS = [consts.tile([128, N_REC], F32, tag=f"sS{h}", name=f"sS{h}") for h in range(2)]
            statsSS = [consts.tile([128, N_REC], F32, tag=f"sSS{h}", name=f"sSS{h}") for h in range(2)]
            scr = consts.tile([128, 500], F16, tag="scr")
            sums = [consts.tile([128, 2], F32, tag=f"sums{h}", name=f"sums{h}") for h in range(2)]
            ag_t = [consts.tile([128, 8, 2], F32, tag=f"ag{h}", name=f"ag{h}") for h in range(2)]
            gh_t = [consts.tile([128, 1], F32, tag=f"gh{h}", name=f"gh{h}") for h in range(2)]
            dl_t = [consts.tile([128, 1], F32, tag=f"dl{h}", name=f"dl{h}") for h in range(2)]
            cc_in = [dr.tile([128, 2], F32, name=f"cci{h}") for h in range(2)]
            cc_out = [dr.tile([8, 128, 2], F32, addr_space="Shared", name=f"cco{h}")
                      for h in range(2)]
            xres = [xres_p.tile([128, TW_PAD], F16, tag=f"xr{b}", name=f"xr{b}") for b in range(B_LOC)]

            def emit_unit(h, b, xt):
                """Pass-1 MM chain + y materialization + fused stats for one
                (half, batch) unit."""
                for bi, (f0, ms) in enumerate(BATCHES):
                    used = sum(ms)
                    nch = len(ms)
                    p1 = ps1.tile([128, 4, 128], F32, tag="p1")
                    co = f0
                    for ci, m in enumerate(ms):
                        nc.tensor.matmul(
                            p1[:, ci, :], xt[:, co:co + 128], wblk_t[h],
                            start=True, stop=True,
                        )
                        co += m
                    y1t = y1tp.tile([128, 4, 128], F16, tag="y1t")
                    nc.scalar.copy(out=y1t[:, :nch, :], in_=p1[:, :nch, :])
                    p2 = ps2.tile([128, 512], F32, tag="p2")
                    co = 0
                    for ci, m in enumerate(ms):
                        for gl in range(2):
                            nc.tensor.matmul(
                                p2[gl * 64:(gl + 1) * 64, co:co + m],
                                y1t[0:m, ci, gl * 64:(gl + 1) * 64],
                                arhs_t[2 * h + gl][:m, :m],
                                start=True, stop=True,
                                tile_position=(0, gl * 64),
                            )
                        co += m
                    yslice = y16[h][:, b, f0:f0 + used]
                    rec = b * N_BAT + bi
                    if bi in ACT_COPYS:
                        nc.scalar.activation(
                            out=yslice, in_=p2[:, :used],
                            func=mybir.ActivationFunctionType.Copy,
                            accum_out=statsS[h][:, rec:rec + 1],
                        )
                    else:
                        nc.vector.tensor_scalar(
                            out=yslice, in0=p2[:, :used],
                            scalar1=1.0, scalar2=0.0,
                            op0=ALU.mult, op1=ALU.add,
                            accum_out=statsS[h][:, rec:rec + 1],
                        )
                    nc.vector.scalar_tensor_tensor(
                        out=scr[:, :used], in0=yslice, scalar=1.0, in1=yslice,
                        op0=ALU.mult, op1=ALU.mult,
                        accum_out=statsSS[h][:, rec:rec + 1],
                    )

            def stats_epilogue(h):
                """Reduce per-record stats, ship to DRAM, AllGather."""
                nc.vector.tensor_reduce(
                    out=sums[h][:, 0:1], in_=statsS[h],
                    axis=mybir.AxisListType.X, op=ALU.add,
                )
                nc.vector.tensor_reduce(
                    out=sums[h][:, 1:2], in_=statsSS[h],
                    axis=mybir.AxisListType.X, op=ALU.add,
                )
                nc.gpsimd.dma_start(out=cc_in[h], in_=sums[h])
                nc.gpsimd.collective_compute(
                    "AllGather", ALU.bypass,
                    replica_groups=[list(range(N_CORES))],
                    ins=[cc_in[h][:, :]], outs=[cc_out[h][:, :, :]],
                )

            def post_ag(h):
                """Combine 8 cores' (S, SS), derive gh = gamma/std and
                dl = beta - gh*mean."""
                for r in range(N_CORES):
                    nc.gpsimd.dma_start(out=ag_t[h][:, r, :], in_=cc_out[h][r])
                a = ag_t[h]
                nc.vector.tensor_add(a[:, 0:4, :], a[:, 0:4, :], a[:, 4:8, :])
                nc.vector.tensor_add(a[:, 0:2, :], a[:, 0:2, :], a[:, 2:4, :])
                nc.vector.tensor_add(a[:, 0:1, :], a[:, 0:1, :], a[:, 1:2, :])
                gmean = consts.tile([128, 1], F32, tag=f"gmean{h}")
                var = consts.tile([128, 1], F32, tag=f"var{h}")
                tmp = consts.tile([128, 1], F32, tag=f"tmpe{h}")
                nc.scalar.mul(out=gmean, in_=a[:, 0, 0:1], mul=1.0 / N_PER_CH)
                nc.scalar.mul(out=var, in_=a[:, 0, 1:2], mul=1.0 / N_PER_CH)
                nc.vector.tensor_mul(tmp, gmean, gmean)
                nc.vector.tensor_sub(var, var, tmp)
                nc.scalar.activation(
                    out=var, in_=var, func=mybir.ActivationFunctionType.Sqrt,
                    bias=eps_t, scale=1.0,
                )
                nc.vector.reciprocal(out=var, in_=var)
                nc.vector.tensor_mul(gh_t[h], gbn_t[h][:, 0:1], var)
                nc.vector.tensor_mul(tmp, gmean, gh_t[h])
                nc.vector.tensor_sub(dl_t[h], gbn_t[h][:, 1:2], tmp)

            def emit_p2(h, b, xsrc, st1_act=False):
                """Pass-2: out = (y*gh + dl) + x for one unit; stage-1 on
                Pool (or Act), stage-2 + out DMA on DVE."""
                tmp = tmp16p.tile([128, TW], F16, tag="tmp")
                if st1_act:
                    nc.scalar.activation(
                        out=tmp, in_=y16[h][:, b, :],
                        func=mybir.ActivationFunctionType.Identity,
                        bias=dl_t[h], scale=gh_t[h],
                    )
                else:
                    nc.gpsimd.tensor_scalar(
                        out=tmp, in0=y16[h][:, b, :],
                        scalar1=gh_t[h], scalar2=dl_t[h],
                        op0=ALU.mult, op1=ALU.add,
                    )
                nc.vector.tensor_add(tmp, tmp, xsrc[:, :TW])
                nc.sync.dma_start(
                    out=out_d[b, h * 128:(h + 1) * 128, :], in_=tmp,
                )

            # ---- pass 1 h0 (x-h1 prefetched into resident tiles) ----
            xt_h0 = []
            for b in range(2):
                xt = xp.tile([128, TW_PAD], F16, tag="xt")
                nc.sync.dma_start(out=xt, in_=x16_in[b, 0])
                xt_h0.append(xt)
            for b in range(B_LOC):
                emit_unit(0, b, xt_h0[b])
                if b + 2 < B_LOC:
                    xt = xp.tile([128, TW_PAD], F16, tag="xt")
                    nc.sync.dma_start(out=xt, in_=x16_in[b + 2, 0])
                    xt_h0.append(xt)
                nc.sync.dma_start(out=xres[b], in_=x16_in[b, 1])
            stats_epilogue(0)

            # ---- pass 1 h1 interleaved with pass 2 h0 ----
            for k in range(B_LOC):
                emit_unit(1, k, xres[k])
                if k == 2:
                    post_ag(0)
                if k >= 3:
                    b = k - 3
                    xt2 = xp.tile([128, TW_PAD], F16, tag="xt")
                    nc.sync.dma_start(out=xt2[:, :TW], in_=x16_in[b, 0, :, :TW])
                    emit_p2(0, b, xt2)
            for b in range(5, B_LOC):
                xt2 = xp.tile([128, TW_PAD], F16, tag="xt")
                nc.sync.dma_start(out=xt2[:, :TW], in_=x16_in[b, 0, :, :TW])
                emit_p2(0, b, xt2)
            stats_epilogue(1)
            post_ag(1)

            # ---- pass 2 h1 (x resident in SBUF) ----
            for b in range(B_LOC):
                emit_p2(1, b, xres[b], st1_act=(b in ACT_ST1))

    nc.finalize()
    return nc


def _prep_consts(A, A_group, conv_w, gamma, beta):
    A_sum = A.sum(axis=0)
    row_sum = np.clip(A_sum.sum(axis=-1, keepdims=True), 1e-6, None)
    A_g = (A_sum / row_sum)[None, :, :] + A_group          # (4,25,25)
    wblk = np.zeros((2, 128, 128), np.float16)
    for h in range(2):
        for gl in range(2):
            g = 2 * h + gl
            wblk[h, gl * 64:(gl + 1) * 64, gl * 64:(gl + 1) * 64] = \
                conv_w[g].T.astype(np.float16)
    eye = np.eye(5, dtype=np.float32)
    arhs = np.stack([np.kron(eye, A_g[g].T) for g in range(G)]).astype(np.float16)
    gbn = np.stack(
        [np.stack([gamma.reshape(2, 128)[h], beta.reshape(2, 128)[h]], axis=1)
         for h in range(2)]
    ).astype(np.float32)
    return wblk, np.ascontiguousarray(arhs), np.ascontiguousarray(gbn)


def _run(inputs, trace=False, **kw):
    if "nc" not in _cache:
        _cache["nc"] = _build()
    nc = _cache["nc"]
    x = np.asarray(inputs["x"], dtype=np.float32)
    wblk, arhs, gbn = _prep_consts(
        np.asarray(inputs["A"], np.float32),
        np.asarray(inputs["A_group"], np.float32),
        np.asarray(inputs["conv_w"], np.float32),
        np.asarray(inputs["gamma"], np.float32),
        np.asarray(inputs["beta"], np.float32),
    )
    xs = x.reshape(N_CORES, B_LOC, 2, 128, TW)
    x16 = np.zeros((N_CORES, B_LOC, 2, 128, TW_PAD), np.float16)
    x16[..., :TW] = xs.astype(np.float16)
    in_maps = [
        {"x16": np.ascontiguousarray(x16[i]), "wblk": wblk, "arhs": arhs, "gbn": gbn}
        for i in range(N_CORES)
    ]
    res = run_bass_kernel_spmd(nc, in_maps, list(range(N_CORES)), trace=trace, **kw)
    out = np.concatenate([res.results[i]["out"][None] for i in range(N_CORES)])
    return out.astype(np.float32).reshape(B, C, T, V), res


def kernel(**inputs) -> np.ndarray:
    out, _ = _run(inputs)
    return out


# revision 8
# speedup vs baseline: 1.3304x; 1.3304x over previous
"""Trainium2 Bass kernel for CTRLightGCN-style GNN message passing block.

Reference computation (per full input):
    A_g = row_normalized(A.sum(0)) + A_group                    # (4,25,25)
    xg = x.reshape(B, 4, 64, T, V)
    y  = einsum('gdc,gvw,bgctw->bgdtv', conv_w, A_g, xg).reshape(B, C, T, V)
    out = x + BN_train(y) * gamma + beta        (BN stats over B,T,V per C)

Strategy (v2): data-parallel over batch B=64 across 8 cores (8 per core).
Per core, per (b, channel-half) the two contractions run as a PE matmul
chain (fp16 inputs, fp32 PSUM accum):

  MM1:  lhsT = x16 chunk (stationary) x wblk  -> y1T chunk in PSUM
  MM2:  lhsT = y1T slice x kron(I_5, A_g^T)   -> y chunk (gd x (t,v)) PSUM

v2 changes vs baseline (251us):
 - fp16 DRAM output (host upcasts): halves pass-2 write traffic.
 - x half-1 tiles stay resident in SBUF through pass 1, so pass-2 h1
   needs no x re-read (saves 6.8MB DMA and shortens the exposed tail).
 - per-channel sums fused into the PSUM->SBUF y copy via accum_out
   (Act/DVE alternating); second moment via one DVE tensor_tensor_reduce
   on the fp16 y slab. No bn_stats pass.
 - cross-core stats combine via AllGather (4.6us floor) + 3 local adds
   instead of AllReduce (~21us); a dummy AllGather at t=0 warms the CC
   stream.
 - pass-2 out = (y*gh + dl) + x runs stage-1 on the otherwise idle Pool
   engine (1-input tensor_scalar, SBUF-only) or Act (Identity act), and
   stage-2 + output DMA on DVE; pass-2 h0 is interleaved into pass-1 h1
   so it hides in the AllGather shadow.
"""
import numpy as np

import concourse.bacc as bacc
import concourse.tile as tile
from concourse import mybir
from concourse.bass_utils import run_bass_kernel_spmd

# ---- problem constants (hardcoded per contract) ----
B, C, T, V = 64, 256, 128, 25
G = 4
N_CORES = 8
B_LOC = B // N_CORES          # 8
TW = T * V                    # 3200
TW_PAD = 3328                 # x16 padded so every 128-col lhsT read is in-bounds
BN_EPS = 1e-5
N_PER_CH = B * TW             # 204800 (global per-channel count)

# chunk = 5 t-rows = 125 cols (last chunk 3 t = 75); batches of 4 chunks -> <=500 cols
CHUNK_M = [125] * 25 + [75]
BATCHES = []                  # list of (f0, [m...]) per (b,h)
_f = 0
_i = 0
while _i < len(CHUNK_M):
    ms = CHUNK_M[_i:_i + 4]
    if sum(ms) > 500:
        ms = CHUNK_M[_i:_i + 2]
    BATCHES.append((_f, ms))
    _f += sum(ms)
    _i += len(ms)
N_BAT = len(BATCHES)          # 7 (6x500 + 1x200)
N_REC = B_LOC * N_BAT         # 56 stat records per half

ACT_COPYS = (1, 2, 4, 5)      # batches whose y copy runs on Act (rest DVE)
F32 = mybir.dt.float32
F16 = mybir.dt.float16
ALU = mybir.AluOpType

_cache = {}


def _build():
    nc = bacc.Bacc()
    x16_in = nc.dram_tensor("x16", [B_LOC, 2, 128, TW_PAD], F16, kind="ExternalInput")
    wblk_in = nc.dram_tensor("wblk", [2, 128, 128], F16, kind="ExternalInput")
    arhs_in = nc.dram_tensor("arhs", [G, 125, 125], F16, kind="ExternalInput")
    gbn_in = nc.dram_tensor("gbn", [2, 128, 2], F32, kind="ExternalInput")
    out_d = nc.dram_tensor("out", [B_LOC, C, TW], F16, kind="ExternalOutput")

    with tile.TileContext(nc) as tc:
        with (
            tc.tile_pool(name="consts", bufs=1) as consts,
            tc.tile_pool(name="resid", bufs=1) as resid,
            tc.tile_pool(name="xres", bufs=1) as xres_p,
            tc.tile_pool(name="xp", bufs=3) as xp,
            tc.tile_pool(name="tmp16", bufs=3) as tmp16p,
            tc.tile_pool(name="y1t", bufs=3) as y1tp,
            tc.tile_pool(name="ps1", bufs=3, space="PSUM") as ps1,
            tc.tile_pool(name="ps2", bufs=4, space="PSUM") as ps2,
            tc.tile_pool(name="psw", bufs=1, space="PSUM") as psw,
            tc.tile_pool(name="dr", bufs=1, space="DRAM") as dr,
        ):
            # ---- PE HAM warmup (~4us of dummy matmuls) ----
            wtile = consts.tile([128, 128], F16, tag="warm")
            nc.vector.memset(wtile, 0.0)
            wp = psw.tile([128, 128], F32, tag="warmp")
            for _ in range(48):
                nc.tensor.matmul(wp, wtile, wtile, start=True, stop=True)
            wsink = consts.tile([128, 1], F32, tag="wsink")
            nc.scalar.copy(out=wsink, in_=wp[:, 0:1])

            # ---- constants ----
            wblk_t = []
            gbn_t = []
            arhs_t = []
            for h in range(2):
                w = consts.tile([128, 128], F16, tag=f"wblk{h}")
                nc.sync.dma_start(out=w, in_=wblk_in[h])
                wblk_t.append(w)
                gbt = consts.tile([128, 2], F32, tag=f"gbn{h}")
                nc.sync.dma_start(out=gbt, in_=gbn_in[h])
                gbn_t.append(gbt)
            for g in range(G):
                a = consts.tile([125, 125], F16, tag=f"arhs{g}")
                nc.sync.dma_start(out=a, in_=arhs_in[g])
                arhs_t.append(a)
            eps_t = consts.tile([128, 1], F32, tag="eps")
            nc.vector.memset(eps_t, BN_EPS)

            y16 = [resid.tile([128, B_LOC, TW], F16, tag=f"y16_{h}", name=f"y16_{h}")
                   for h in range(2)]
            statsbuf = [consts.tile([128, N_REC, 6], F32, tag=f"sb{h}", name=f"sb{h}")
                        for h in range(2)]
            sums = [consts.tile([128, 2], F32, tag=f"sums{h}", name=f"sums{h}") for h in range(2)]
            ag_t = [consts.tile([128, 8, 2], F32, tag=f"ag{h}", name=f"ag{h}") for h in range(2)]
            gh_t = [consts.tile([128, 1], F32, tag=f"gh{h}", name=f"gh{h}") for h in range(2)]
            dl_t = [consts.tile([128, 1], F32, tag=f"dl{h}", name=f"dl{h}") for h in range(2)]
            cc_in = [dr.tile([128, 2], F32, name=f"cci{h}") for h in range(2)]
            cc_out = [dr.tile([8, 128, 2], F32, addr_space="Shared", name=f"cco{h}")
                      for h in range(2)]
            xres = [xres_p.tile([128, TW_PAD], F16, tag=f"xr{b}", name=f"xr{b}") for b in range(B_LOC)]

            def emit_unit(h, b, xt):
                """Pass-1 MM chain + y materialization + fused stats for one
                (half, batch) unit."""
                for bi, (f0, ms) in enumerate(BATCHES):
                    used = sum(ms)
                    nch = len(ms)
                    p1 = ps1.tile([128, 4, 128], F32, tag="p1")
                    co = f0
                    for ci, m in enumerate(ms):
                        nc.tensor.matmul(
                            p1[:, ci, :], xt[:, co:co + 128], wblk_t[h],
                            start=True, stop=True,
                        )
                        co += m
                    y1t = y1tp.tile([128, 4, 128], F16, tag="y1t")
                    nc.scalar.copy(out=y1t[:, :nch, :], in_=p1[:, :nch, :])
                    p2 = ps2.tile([128, 512], F32, tag="p2")
                    co = 0
                    for ci, m in enumerate(ms):
                        for gl in range(2):
                            nc.tensor.matmul(
                                p2[gl * 64:(gl + 1) * 64, co:co + m],
                                y1t[0:m, ci, gl * 64:(gl + 1) * 64],
                                arhs_t[2 * h + gl][:m, :m],
                                start=True, stop=True,
                                tile_position=(0, gl * 64),
                            )
                        co += m
                    yslice = y16[h][:, b, f0:f0 + used]
                    rec = b * N_BAT + bi
                    if bi in ACT_COPYS:
                        nc.scalar.copy(out=yslice, in_=p2[:, :used])
                    else:
                        nc.vector.tensor_copy(out=yslice, in_=p2[:, :used])
                    nc.vector.bn_stats(
                        out=statsbuf[h][:, rec, :], in_=yslice
                    )

            def stats_epilogue(h):
                """Aggregate bn_stats records -> local (S, SS), ship, gather."""
                n_loc = float(B_LOC * TW)
                mv = consts.tile([128, 2], F32, tag=f"mv{h}", name=f"mv{h}")
                nc.vector.bn_aggr(out=mv, in_=statsbuf[h])
                m2 = consts.tile([128, 1], F32, tag=f"m2{h}", name=f"m2{h}")
                nc.vector.tensor_mul(m2, mv[:, 0:1], mv[:, 0:1])
                nc.vector.tensor_add(m2, m2, mv[:, 1:2])
                nc.scalar.mul(out=sums[h][:, 0:1], in_=mv[:, 0:1], mul=n_loc)
                nc.scalar.mul(out=sums[h][:, 1:2], in_=m2, mul=n_loc)
                nc.gpsimd.dma_start(out=cc_in[h], in_=sums[h])
                nc.gpsimd.collective_compute(
                    "AllGather", ALU.bypass,
                    replica_groups=[list(range(N_CORES))],
                    ins=[cc_in[h][:, :]], outs=[cc_out[h][:, :, :]],
                )

            def post_ag(h):
                """Combine 8 cores' (S, SS), derive gh = gamma/std and
                dl = beta - gh*mean."""
                for r in range(N_CORES):
                    nc.gpsimd.dma_start(out=ag_t[h][:, r, :], in_=cc_out[h][r])
                a = ag_t[h]
                nc.vector.tensor_add(a[:, 0:4, :], a[:, 0:4, :], a[:, 4:8, :])
                nc.vector.tensor_add(a[:, 0:2, :], a[:, 0:2, :], a[:, 2:4, :])
                nc.vector.tensor_add(a[:, 0:1, :], a[:, 0:1, :], a[:, 1:2, :])
                gmean = consts.tile([128, 1], F32, tag=f"gmean{h}")
                var = consts.tile([128, 1], F32, tag=f"var{h}")
                tmp = consts.tile([128, 1], F32, tag=f"tmpe{h}")
                nc.scalar.mul(out=gmean, in_=a[:, 0, 0:1], mul=1.0 / N_PER_CH)
                nc.scalar.mul(out=var, in_=a[:, 0, 1:2], mul=1.0 / N_PER_CH)
                nc.vector.tensor_mul(tmp, gmean, gmean)
                nc.vector.tensor_sub(var, var, tmp)
                nc.scalar.activation(
                    out=var, in_=var, func=mybir.ActivationFunctionType.Sqrt,
                    bias=eps_t, scale=1.0,
                )
                nc.vector.reciprocal(out=var, in_=var)
                nc.vector.tensor_mul(gh_t[h], gbn_t[h][:, 0:1], var)
                nc.vector.tensor_mul(tmp, gmean, gh_t[h])
                nc.vector.tensor_sub(dl_t[h], gbn_t[h][:, 1:2], tmp)

            def emit_p2(h, b, xsrc, st1="pool"):
                """Pass-2: out = (y*gh + dl) + x for one unit; stage-1 on
                Pool/Act/DVE, stage-2 + out DMA on DVE."""
                tmp = tmp16p.tile([128, TW], F16, tag="tmp")
                if st1 == "act":
                    nc.scalar.activation(
                        out=tmp, in_=y16[h][:, b, :],
                        func=mybir.ActivationFunctionType.Identity,
                        bias=dl_t[h], scale=gh_t[h],
                    )
                elif st1 == "dve":
                    nc.vector.tensor_scalar(
                        out=tmp, in0=y16[h][:, b, :],
                        scalar1=gh_t[h], scalar2=dl_t[h],
                        op0=ALU.mult, op1=ALU.add,
                    )
                else:
                    nc.gpsimd.tensor_scalar(
                        out=tmp, in0=y16[h][:, b, :],
                        scalar1=gh_t[h], scalar2=dl_t[h],
                        op0=ALU.mult, op1=ALU.add,
                    )
                nc.vector.tensor_add(tmp, tmp, xsrc[:, :TW])
                nc.sync.dma_start(
                    out=out_d[b, h * 128:(h + 1) * 128, :], in_=tmp,
                )

            # ---- pass 1 h0 (x-h1 prefetched into resident tiles) ----
            xt_h0 = []
            for b in range(2):
                xt = xp.tile([128, TW_PAD], F16, tag="xt")
                nc.sync.dma_start(out=xt, in_=x16_in[b, 0])
                xt_h0.append(xt)
            for b in range(B_LOC):
                emit_unit(0, b, xt_h0[b])
                if b + 2 < B_LOC:
                    xt = xp.tile([128, TW_PAD], F16, tag="xt")
                    nc.sync.dma_start(out=xt, in_=x16_in[b + 2, 0])
                    xt_h0.append(xt)
                nc.sync.dma_start(out=xres[b], in_=x16_in[b, 1])
            stats_epilogue(0)

            # ---- pass 1 h1 interleaved with pass 2 h0 ----
            for k in range(B_LOC):
                emit_unit(1, k, xres[k])
                if k == 2:
                    post_ag(0)
                if k >= 3:
                    b = k - 3
                    xt2 = xp.tile([128, TW_PAD], F16, tag="xt")
                    nc.sync.dma_start(out=xt2[:, :TW], in_=x16_in[b, 0, :, :TW])
                    emit_p2(0, b, xt2)
            for b in range(5, B_LOC):
                xt2 = xp.tile([128, TW_PAD], F16, tag="xt")
                nc.sync.dma_start(out=xt2[:, :TW], in_=x16_in[b, 0, :, :TW])
                emit_p2(0, b, xt2)
            stats_epilogue(1)
            post_ag(1)

            # ---- pass 2 h1 (x resident in SBUF) ----
            ST1_H1 = {0: "act", 3: "act", 6: "act", 1: "pool", 4: "pool",
                      7: "pool", 2: "dve", 5: "dve"}
            for b in range(B_LOC):
                emit_p2(1, b, xres[b], st1=ST1_H1[b])

    nc.finalize()
    return nc


def _prep_consts(A, A_group, conv_w, gamma, beta):
    A_sum = A.sum(axis=0)
    row_sum = np.clip(A_sum.sum(axis=-1, keepdims=True), 1e-6, None)
    A_g = (A_sum / row_sum)[None, :, :] + A_group          # (4,25,25)
    wblk = np.zeros((2, 128, 128), np.float16)
    for h in range(2):
        for gl in range(2):
            g = 2 * h + gl
            wblk[h, gl * 64:(gl + 1) * 64, gl * 64:(gl + 1) * 64] = \
                conv_w[g].T.astype(np.float16)
    eye = np.eye(5, dtype=np.float32)
    arhs = np.stack([np.kron(eye, A_g[g].T) for g in range(G)]).astype(np.float16)
    gbn = np.stack(
        [np.stack([gamma.reshape(2, 128)[h], beta.reshape(2, 128)[h]], axis=1)
         for h in range(2)]
    ).astype(np.float32)
    return wblk, np.ascontiguousarray(arhs), np.ascontiguousarray(gbn)


def _run(inputs, trace=False, **kw):
    if "nc" not in _cache:
        _cache["nc"] = _build()
    nc = _cache["nc"]
    x = np.asarray(inputs["x"], dtype=np.float32)
    wblk, arhs, gbn = _prep_consts(
        np.asarray(inputs["A"], np.float32),
        np.asarray(inputs["A_group"], np.float32),
        np.asarray(inputs["conv_w"], np.float32),
        np.asarray(inputs["gamma"], np.float32),
        np.asarray(inputs["beta"], np.float32),
    )
    xs = x.reshape(N_CORES, B_LOC, 2, 128, TW)
    x16 = np.zeros((N_CORES, B_LOC, 2, 128, TW_PAD), np.float16)
    x16[..., :TW] = xs.astype(np.float16)
    in_maps = [
        {"x16": np.ascontiguousarray(x16[i]), "wblk": wblk, "arhs": arhs, "gbn": gbn}
        for i in range(N_CORES)
    ]
    res = run_bass_kernel_spmd(nc, in_maps, list(range(N_CORES)), trace=trace, **kw)
    out = np.concatenate([res.results[i]["out"][None] for i in range(N_CORES)])
    return out.astype(np.float32).reshape(B, C, T, V), res


def kernel(**inputs) -> np.ndarray:
    out, _ = _run(inputs)
    return out


# revision 10
# speedup vs baseline: 1.4278x; 1.0732x over previous
"""Trainium2 Bass kernel for CTRLightGCN-style GNN message passing block.

Reference computation (per full input):
    A_g = row_normalized(A.sum(0)) + A_group                    # (4,25,25)
    xg = x.reshape(B, 4, 64, T, V)
    y  = einsum('gdc,gvw,bgctw->bgdtv', conv_w, A_g, xg).reshape(B, C, T, V)
    out = x + BN_train(y) * gamma + beta        (BN stats over B,T,V per C)

Strategy (v4): data-parallel over batch B=64 across 8 cores (8 per core).
Per core, per (b, channel-half) unit the two contractions run as a PE
matmul chain (fp16 operands, fp32 PSUM):

  MM1:  lhsT = x16 chunk (stationary) x wblk  -> y1T chunk in PSUM
  MM2:  lhsT = y1T slice x kron(I_5, A_g^T)   -> y chunk (gd x (t,v)) PSUM

Pass-1 work is grouped into 1000-column super-batches (2 PSUM banks)
so each PSUM->SBUF copy is one big instruction; the critical resource
is Act+DVE throughput (the only PSUM readers).  BN stats come from
per-500-col bn_stats records (one read gives sum and sumsq), aggregated
by bn_aggr, combined across cores with an AllGather (~11us warm) plus
three local adds.  fp16 DRAM output (host upcasts); x half-1 stays
resident in SBUF so the post-AllGather tail is write-only.  Pass-2
out = (y*gh + dl) + x runs stage-1 on Pool/DVE, stage-2 on DVE (fp16
2x), interleaved under pass-1 h1 and the AllGather shadows.
"""
import numpy as np

import concourse.bacc as bacc
import concourse.tile as tile
from concourse import mybir
from concourse.bass_utils import run_bass_kernel_spmd

# ---- problem constants (hardcoded per contract) ----
B, C, T, V = 64, 256, 128, 25
G = 4
N_CORES = 8
B_LOC = B // N_CORES          # 8
TW = T * V                    # 3200
TW_PAD = 3328                 # x16 padded so every 128-col lhsT read is in-bounds
BN_EPS = 1e-5
N_PER_CH = B * TW             # 204800 (global per-channel count)

# chunks of 5 t-rows (125 cols; last chunk 3 t-rows = 75).  Super-batches
# of 8 chunks -> 1000 cols (2 PSUM banks); last super = 2 chunks (200).
CHUNK_M = [125] * 25 + [75]
SUPERS = []                   # (f0, [m...]) per super-batch
_f = 0
for _i in range(0, len(CHUNK_M), 8):
    ms = CHUNK_M[_i:_i + 8]
    SUPERS.append((_f, ms))
    _f += sum(ms)
N_REC = B_LOC * 7             # bn_stats records per half (2+2+2+1 per unit)

F32 = mybir.dt.float32
F16 = mybir.dt.float16
ALU = mybir.AluOpType

_cache = {}


def _build():
    nc = bacc.Bacc()
    x16_in = nc.dram_tensor("x16", [B_LOC, 2, 128, TW_PAD], F16, kind="ExternalInput")
    wblk_in = nc.dram_tensor("wblk", [2, 128, 128], F16, kind="ExternalInput")
    arhs_in = nc.dram_tensor("arhs", [G, 125, 125], F16, kind="ExternalInput")
    gbn_in = nc.dram_tensor("gbn", [2, 128, 2], F32, kind="ExternalInput")
    out_d = nc.dram_tensor("out", [B_LOC, C, TW], F16, kind="ExternalOutput")

    with tile.TileContext(nc) as tc:
        with (
            tc.tile_pool(name="consts", bufs=1) as consts,
            tc.tile_pool(name="resid", bufs=1) as resid,
            tc.tile_pool(name="xres", bufs=1) as xres_p,
            tc.tile_pool(name="xp", bufs=3) as xp,
            tc.tile_pool(name="tmp16", bufs=3) as tmp16p,
            tc.tile_pool(name="y1t", bufs=3) as y1tp,
            tc.tile_pool(name="ps1", bufs=2, space="PSUM") as ps1,
            tc.tile_pool(name="ps2", bufs=2, space="PSUM") as ps2,
            tc.tile_pool(name="dr", bufs=1, space="DRAM") as dr,
        ):
            # ---- PE HAM warmup (~4us of dummy matmuls) ----
            wtile = consts.tile([128, 128], F16, tag="warm")
            nc.vector.memset(wtile, 0.0)
            wp = ps1.tile([128, 8, 128], F32, tag="p1")
            for _ in range(48):
                nc.tensor.matmul(wp[:, 0, :], wtile, wtile, start=True, stop=True)
            wsink = consts.tile([128, 1], F32, tag="wsink")
            nc.scalar.copy(out=wsink, in_=wp[:, 0, 0:1])

            # ---- constants ----
            wblk_t = []
            gbn_t = []
            arhs_t = []
            for h in range(2):
                w = consts.tile([128, 128], F16, tag=f"wblk{h}")
                nc.sync.dma_start(out=w, in_=wblk_in[h])
                wblk_t.append(w)
                gbt = consts.tile([128, 2], F32, tag=f"gbn{h}")
                nc.sync.dma_start(out=gbt, in_=gbn_in[h])
                gbn_t.append(gbt)
            for g in range(G):
                a = consts.tile([125, 125], F16, tag=f"arhs{g}")
                nc.sync.dma_start(out=a, in_=arhs_in[g])
                arhs_t.append(a)
            eps_t = consts.tile([128, 1], F32, tag="eps")
            nc.vector.memset(eps_t, BN_EPS)

            y16 = [resid.tile([128, B_LOC, TW], F16, tag=f"y16_{h}", name=f"y16_{h}")
                   for h in range(2)]
            statsbuf = [consts.tile([128, N_REC, 6], F32, tag=f"sb{h}", name=f"sb{h}")
                        for h in range(2)]
            sums = [consts.tile([128, 2], F32, tag=f"sums{h}", name=f"sums{h}")
                    for h in range(2)]
            ag_t = [consts.tile([128, 8, 2], F32, tag=f"ag{h}", name=f"ag{h}")
                    for h in range(2)]
            gh_t = [consts.tile([128, 1], F32, tag=f"gh{h}", name=f"gh{h}")
                    for h in range(2)]
            dl_t = [consts.tile([128, 1], F32, tag=f"dl{h}", name=f"dl{h}")
                    for h in range(2)]
            cc_in = [dr.tile([128, 2], F32, name=f"cci{h}") for h in range(2)]
            cc_out = [dr.tile([8, 128, 2], F32, addr_space="Shared", name=f"cco{h}")
                      for h in range(2)]
            xres = [xres_p.tile([128, TW_PAD], F16, tag=f"xr{b}", name=f"xr{b}")
                    for b in range(B_LOC)]

            def emit_unit(h, b, xt):
                """Pass-1 MM chain + y materialization + stats for one unit."""
                rec = b * 7
                for si, (f0, ms) in enumerate(SUPERS):
                    used = sum(ms)
                    nch = len(ms)
                    p1 = ps1.tile([128, 8, 128], F32, tag="p1")
                    co = f0
                    for ci, m in enumerate(ms):
                        nc.tensor.matmul(
                            p1[:, ci, :], xt[:, co:co + 128], wblk_t[h],
                            start=True, stop=True,
                        )
                        co += m
                    y1t = y1tp.tile([128, 8, 128], F16, tag="y1t")
                    nc.scalar.copy(out=y1t[:, :nch, :], in_=p1[:, :nch, :])
                    # two 512-col banks: chunks 0-3 in bank 0, 4-7 in bank 1
                    # (a flat 1000-col tile would cross a bank mid-chunk)
                    p2 = ps2.tile([128, 2, 512], F32, tag="p2")
                    for ci, m in enumerate(ms):
                        half = ci // 4
                        off = (ci % 4) * 125
                        for gl in range(2):
                            nc.tensor.matmul(
                                p2[gl * 64:(gl + 1) * 64, half, off:off + m],
                                y1t[0:m, ci, gl * 64:(gl + 1) * 64],
                                arhs_t[2 * h + gl][:m, :m],
                                start=True, stop=True,
                                tile_position=(0, gl * 64),
                            )
                    yslice = y16[h][:, b, f0:f0 + used]
                    src = p2[:, :, :500] if used == 1000 else p2[:, 0:1, :used]
                    if si % 2 == 0:
                        nc.scalar.copy(out=yslice, in_=src)
                    else:
                        nc.vector.tensor_copy(out=yslice, in_=src)
                    for hw in range(2 if used > 500 else 1):
                        wn = min(500, used - hw * 500)
                        nc.vector.bn_stats(
                            out=statsbuf[h][:, rec, :], in_=p2[:, hw, :wn]
                        )
                        rec += 1

            def stats_epilogue(h):
                """Aggregate bn_stats records -> local (S, SS), ship, gather."""
                n_loc = float(B_LOC * TW)
                mv = consts.tile([128, 2], F32, tag=f"mv{h}", name=f"mv{h}")
                nc.vector.bn_aggr(out=mv, in_=statsbuf[h])
                m2 = consts.tile([128, 1], F32, tag=f"m2{h}", name=f"m2{h}")
                nc.vector.tensor_mul(m2, mv[:, 0:1], mv[:, 0:1])
                nc.vector.tensor_add(m2, m2, mv[:, 1:2])
                nc.scalar.mul(out=sums[h][:, 0:1], in_=mv[:, 0:1], mul=n_loc)
                nc.scalar.mul(out=sums[h][:, 1:2], in_=m2, mul=n_loc)
                nc.gpsimd.dma_start(out=cc_in[h], in_=sums[h])
                nc.gpsimd.collective_compute(
                    "AllGather", ALU.bypass,
                    replica_groups=[list(range(N_CORES))],
                    ins=[cc_in[h][:, :]], outs=[cc_out[h][:, :, :]],
                )

            def post_ag(h):
                """Combine 8 cores' (S, SS); gh = gamma/std, dl = beta-gh*mean."""
                for r in range(N_CORES):
                    nc.gpsimd.dma_start(out=ag_t[h][:, r, :], in_=cc_out[h][r])
                a = ag_t[h]
                nc.vector.tensor_add(a[:, 0:4, :], a[:, 0:4, :], a[:, 4:8, :])
                nc.vector.tensor_add(a[:, 0:2, :], a[:, 0:2, :], a[:, 2:4, :])
                nc.vector.tensor_add(a[:, 0:1, :], a[:, 0:1, :], a[:, 1:2, :])
                gmean = consts.tile([128, 1], F32, tag=f"gmean{h}")
                var = consts.tile([128, 1], F32, tag=f"var{h}")
                tmp = consts.tile([128, 1], F32, tag=f"tmpe{h}")
                nc.scalar.mul(out=gmean, in_=a[:, 0, 0:1], mul=1.0 / N_PER_CH)
                nc.scalar.mul(out=var, in_=a[:, 0, 1:2], mul=1.0 / N_PER_CH)
                nc.vector.tensor_mul(tmp, gmean, gmean)
                nc.vector.tensor_sub(var, var, tmp)
                nc.scalar.activation(
                    out=var, in_=var, func=mybir.ActivationFunctionType.Sqrt,
                    bias=eps_t, scale=1.0,
                )
                nc.vector.reciprocal(out=var, in_=var)
                nc.vector.tensor_mul(gh_t[h], gbn_t[h][:, 0:1], var)
                nc.vector.tensor_mul(tmp, gmean, gh_t[h])
                nc.vector.tensor_sub(dl_t[h], gbn_t[h][:, 1:2], tmp)

            def emit_p2(h, b, xsrc, st1="pool"):
                """Pass-2: out = (y*gh + dl) + x; stage-2 + out DMA on DVE."""
                tmp = tmp16p.tile([128, TW], F16, tag="tmp")
                if st1 == "dve":
                    nc.vector.tensor_scalar(
                        out=tmp, in0=y16[h][:, b, :],
                        scalar1=gh_t[h], scalar2=dl_t[h],
                        op0=ALU.mult, op1=ALU.add,
                    )
                elif st1 == "act":
                    nc.scalar.activation(
                        out=tmp, in_=y16[h][:, b, :],
                        func=mybir.ActivationFunctionType.Identity,
                        bias=dl_t[h], scale=gh_t[h],
                    )
                else:
                    nc.gpsimd.tensor_scalar(
                        out=tmp, in0=y16[h][:, b, :],
                        scalar1=gh_t[h], scalar2=dl_t[h],
                        op0=ALU.mult, op1=ALU.add,
                    )
                nc.vector.tensor_add(tmp, tmp, xsrc[:, :TW])
                nc.sync.dma_start(
                    out=out_d[b, h * 128:(h + 1) * 128, :], in_=tmp,
                )

            # ---- pass 1 h0 (x-h1 prefetched into resident tiles) ----
            xt_h0 = []
            for b in range(2):
                xt = xp.tile([128, TW_PAD], F16, tag="xt")
                nc.sync.dma_start(out=xt, in_=x16_in[b, 0])
                xt_h0.append(xt)
            for b in range(B_LOC):
                emit_unit(0, b, xt_h0[b])
                if b + 2 < B_LOC:
                    xt = xp.tile([128, TW_PAD], F16, tag="xt")
                    nc.sync.dma_start(out=xt, in_=x16_in[b + 2, 0])
                    xt_h0.append(xt)
                nc.sync.dma_start(out=xres[b], in_=x16_in[b, 1])
            stats_epilogue(0)

            # pass-2 h0 x re-reads can start as soon as the xp pool frees
            xt2 = []
            for b in range(3):
                t = xp.tile([128, TW_PAD], F16, tag="xt")
                nc.sync.dma_start(out=t[:, :TW], in_=x16_in[b, 0, :, :TW])
                xt2.append(t)

            # ---- pass 1 h1 interleaved with pass 2 h0 ----
            for k in range(B_LOC):
                emit_unit(1, k, xres[k])
                if k == 5:
                    post_ag(0)
                if k >= 6:
                    for b in (3 * (k - 6), 3 * (k - 6) + 1, 3 * (k - 6) + 2):
                        emit_p2(0, b, xt2[b])
                        if b + 3 < B_LOC:
                            t = xp.tile([128, TW_PAD], F16, tag="xt")
                            nc.sync.dma_start(
                                out=t[:, :TW], in_=x16_in[b + 3, 0, :, :TW]
                            )
                            xt2.append(t)
            for b in range(6, B_LOC):
                emit_p2(0, b, xt2[b])
            stats_epilogue(1)
            post_ag(1)

            # ---- pass 2 h1 (x resident in SBUF, write-only tail) ----
            ST1_H1 = {0: "dve", 1: "pool", 2: "dve", 3: "pool", 4: "dve",
                      5: "pool", 6: "dve", 7: "pool"}
            for b in range(B_LOC):
                emit_p2(1, b, xres[b], st1=ST1_H1[b])

    nc.finalize()
    return nc


def _prep_consts(A, A_group, conv_w, gamma, beta):
    A_sum = A.sum(axis=0)
    row_sum = np.clip(A_sum.sum(axis=-1, keepdims=True), 1e-6, None)
    A_g = (A_sum / row_sum)[None, :, :] + A_group          # (4,25,25)
    wblk = np.zeros((2, 128, 128), np.float16)
    for h in range(2):
        for gl in range(2):
            g = 2 * h + gl
            wblk[h, gl * 64:(gl + 1) * 64, gl * 64:(gl + 1) * 64] = \
                conv_w[g].T.astype(np.float16)
    eye = np.eye(5, dtype=np.float32)
    arhs = np.stack([np.kron(eye, A_g[g].T) for g in range(G)]).astype(np.float16)
    gbn = np.stack(
        [np.stack([gamma.reshape(2, 128)[h], beta.reshape(2, 128)[h]], axis=1)
         for h in range(2)]
    ).astype(np.float32)
    return wblk, np.ascontiguousarray(arhs), np.ascontiguousarray(gbn)


def _run(inputs, trace=False, **kw):
    if "nc" not in _cache:
        _cache["nc"] = _build()
    nc = _cache["nc"]
    x = np.asarray(inputs["x"], dtype=np.float32)
    wblk, arhs, gbn = _prep_consts(
        np.asarray(inputs["A"], np.float32),
        np.asarray(inputs["A_group"], np.float32),
        np.asarray(inputs["conv_w"], np.float32),
        np.asarray(inputs["gamma"], np.float32),
        np.asarray(inputs["beta"], np.float32),
    )
    xs = x.reshape(N_CORES, B_LOC, 2, 128, TW)
    x16 = np.zeros((N_CORES, B_LOC, 2, 128, TW_PAD), np.float16)
    x16[..., :TW] = xs.astype(np.float16)
    in_maps = [
        {"x16": np.ascontiguousarray(x16[i]), "wblk": wblk, "arhs": arhs, "gbn": gbn}
        for i in range(N_CORES)
    ]
    res = run_bass_kernel_spmd(nc, in_maps, list(range(N_CORES)), trace=trace, **kw)
    out = np.concatenate([res.results[i]["out"][None] for i in range(N_CORES)])
    return out.astype(np.float32).reshape(B, C, T, V), res


def kernel(**inputs) -> np.ndarray:
    out, _ = _run(inputs)
    return out


# revision 11
# speedup vs baseline: 1.5185x; 1.0635x over previous
"""Trainium2 Bass kernel for CTRLightGCN-style GNN message passing block.

Reference computation (per full input):
    A_g = row_normalized(A.sum(0)) + A_group                    # (4,25,25)
    xg = x.reshape(B, 4, 64, T, V)
    y  = einsum('gdc,gvw,bgctw->bgdtv', conv_w, A_g, xg).reshape(B, C, T, V)
    out = x + BN_train(y) * gamma + beta        (BN stats over B,T,V per C)

Strategy (v4): data-parallel over batch B=64 across 8 cores (8 per core).
Per core, per (b, channel-half) unit the two contractions run as a PE
matmul chain (fp16 operands, fp32 PSUM):

  MM1:  lhsT = x16 chunk (stationary) x wblk  -> y1T chunk in PSUM
  MM2:  lhsT = y1T slice x kron(I_5, A_g^T)   -> y chunk (gd x (t,v)) PSUM

Pass-1 work is grouped into 1000-column super-batches (2 PSUM banks)
so each PSUM->SBUF copy is one big instruction; the critical resource
is Act+DVE throughput (the only PSUM readers).  BN stats come from
per-500-col bn_stats records (one read gives sum and sumsq), aggregated
by bn_aggr, combined across cores with an AllGather (~11us warm) plus
three local adds.  fp16 DRAM output (host upcasts); x half-1 stays
resident in SBUF so the post-AllGather tail is write-only.  Pass-2
out = (y*gh + dl) + x runs stage-1 on Pool/DVE, stage-2 on DVE (fp16
2x), interleaved under pass-1 h1 and the AllGather shadows.
"""
import numpy as np

import concourse.bacc as bacc
import concourse.tile as tile
from concourse import mybir
from concourse.bass_utils import run_bass_kernel_spmd

# ---- problem constants (hardcoded per contract) ----
B, C, T, V = 64, 256, 128, 25
G = 4
N_CORES = 8
B_LOC = B // N_CORES          # 8
TW = T * V                    # 3200
TW_PAD = 3328                 # x16 padded so every 128-col lhsT read is in-bounds
BN_EPS = 1e-5
N_PER_CH = B * TW             # 204800 (global per-channel count)

# chunks of 5 t-rows (125 cols; last chunk 3 t-rows = 75).  Super-batches
# of 8 chunks -> 1000 cols (2 PSUM banks); last super = 2 chunks (200).
CHUNK_M = [125] * 25 + [75]
SUPERS = []                   # (f0, [m...]) per super-batch
_f = 0
for _i in range(0, len(CHUNK_M), 8):
    ms = CHUNK_M[_i:_i + 8]
    SUPERS.append((_f, ms))
    _f += sum(ms)
N_REC = B_LOC * 7             # bn_stats records per half (2+2+2+1 per unit)

F32 = mybir.dt.float32
F16 = mybir.dt.float16
ALU = mybir.AluOpType

_cache = {}


def _build():
    nc = bacc.Bacc()
    x16_in = nc.dram_tensor("x16", [B_LOC, 2, 128, TW_PAD], F16, kind="ExternalInput")
    wblk_in = nc.dram_tensor("wblk", [2, 128, 128], F16, kind="ExternalInput")
    arhs_in = nc.dram_tensor("arhs", [G, 125, 125], F16, kind="ExternalInput")
    gbn_in = nc.dram_tensor("gbn", [2, 128, 2], F32, kind="ExternalInput")
    out_d = nc.dram_tensor("out", [B_LOC, C, TW], F16, kind="ExternalOutput")

    with tile.TileContext(nc) as tc:
        with (
            tc.tile_pool(name="consts", bufs=1) as consts,
            tc.tile_pool(name="resid", bufs=1) as resid,
            tc.tile_pool(name="xres", bufs=1) as xres_p,
            tc.tile_pool(name="xp", bufs=3) as xp,
            tc.tile_pool(name="tmp16", bufs=3) as tmp16p,
            tc.tile_pool(name="y1t", bufs=3) as y1tp,
            tc.tile_pool(name="ps1", bufs=2, space="PSUM") as ps1,
            tc.tile_pool(name="ps2", bufs=2, space="PSUM") as ps2,
            tc.tile_pool(name="dr", bufs=1, space="DRAM") as dr,
        ):
            # ---- PE HAM warmup (~4us of dummy matmuls) ----
            wtile = consts.tile([128, 128], F16, tag="warm")
            nc.vector.memset(wtile, 0.0)
            wp = ps1.tile([128, 8, 128], F32, tag="p1")
            for _ in range(48):
                nc.tensor.matmul(wp[:, 0, :], wtile, wtile, start=True, stop=True)
            wsink = consts.tile([128, 1], F32, tag="wsink")
            nc.scalar.copy(out=wsink, in_=wp[:, 0, 0:1])

            # ---- constants ----
            wblk_t = []
            gbn_t = []
            arhs_t = []
            for h in range(2):
                w = consts.tile([128, 128], F16, tag=f"wblk{h}")
                nc.sync.dma_start(out=w, in_=wblk_in[h])
                wblk_t.append(w)
                gbt = consts.tile([128, 2], F32, tag=f"gbn{h}")
                nc.sync.dma_start(out=gbt, in_=gbn_in[h])
                gbn_t.append(gbt)
            for g in range(G):
                a = consts.tile([125, 125], F16, tag=f"arhs{g}")
                nc.sync.dma_start(out=a, in_=arhs_in[g])
                arhs_t.append(a)
            eps_t = consts.tile([128, 1], F32, tag="eps")
            nc.vector.memset(eps_t, BN_EPS)

            y16 = [resid.tile([128, B_LOC, TW], F16, tag=f"y16_{h}", name=f"y16_{h}")
                   for h in range(2)]
            statsbuf = [consts.tile([128, N_REC, 6], F32, tag=f"sb{h}", name=f"sb{h}")
                        for h in range(2)]
            sums = [consts.tile([128, 2], F32, tag=f"sums{h}", name=f"sums{h}")
                    for h in range(2)]
            ag_t = [consts.tile([128, 8, 2], F32, tag=f"ag{h}", name=f"ag{h}")
                    for h in range(2)]
            gh_t = [consts.tile([128, 1], F32, tag=f"gh{h}", name=f"gh{h}")
                    for h in range(2)]
            dl_t = [consts.tile([128, 1], F32, tag=f"dl{h}", name=f"dl{h}")
                    for h in range(2)]
            cc_in = [dr.tile([128, 2], F32, name=f"cci{h}") for h in range(2)]
            cc_out = [dr.tile([8, 128, 2], F32, addr_space="Shared", name=f"cco{h}")
                      for h in range(2)]
            xres = [xres_p.tile([128, TW_PAD], F16, tag=f"xr{b}", name=f"xr{b}")
                    for b in range(B_LOC)]

            def emit_unit(h, b, xt):
                """Pass-1 MM chain + y materialization + stats for one unit."""
                rec = b * 7
                for si, (f0, ms) in enumerate(SUPERS):
                    used = sum(ms)
                    nch = len(ms)
                    p1 = ps1.tile([128, 8, 128], F32, tag="p1")
                    co = f0
                    for ci, m in enumerate(ms):
                        nc.tensor.matmul(
                            p1[:, ci, :], xt[:, co:co + 128], wblk_t[h],
                            start=True, stop=True,
                        )
                        co += m
                    y1t = y1tp.tile([128, 8, 128], F16, tag="y1t")
                    nc.scalar.copy(out=y1t[:, :nch, :], in_=p1[:, :nch, :])
                    # two 512-col banks: chunks 0-3 in bank 0, 4-7 in bank 1
                    # (a flat 1000-col tile would cross a bank mid-chunk)
                    p2 = ps2.tile([128, 2, 512], F32, tag="p2")
                    for ci, m in enumerate(ms):
                        half = ci // 4
                        off = (ci % 4) * 125
                        for gl in range(2):
                            nc.tensor.matmul(
                                p2[gl * 64:(gl + 1) * 64, half, off:off + m],
                                y1t[0:m, ci, gl * 64:(gl + 1) * 64],
                                arhs_t[2 * h + gl][:m, :m],
                                start=True, stop=True,
                                tile_position=(0, gl * 64),
                            )
                    yslice = y16[h][:, b, f0:f0 + used]
                    src = p2[:, :, :500] if used == 1000 else p2[:, 0:1, :used]
                    if si % 2 == 0:
                        nc.scalar.copy(out=yslice, in_=src)
                    else:
                        nc.vector.tensor_copy(out=yslice, in_=src)
                    for w0 in range(0, used, 500):
                        wn = min(500, used - w0)
                        nc.vector.bn_stats(
                            out=statsbuf[h][:, rec, :],
                            in_=y16[h][:, b, f0 + w0:f0 + w0 + wn],
                        )
                        rec += 1

            def stats_epilogue(h):
                """Aggregate bn_stats records -> local (S, SS), ship, gather."""
                n_loc = float(B_LOC * TW)
                mv = consts.tile([128, 2], F32, tag=f"mv{h}", name=f"mv{h}")
                nc.vector.bn_aggr(out=mv, in_=statsbuf[h])
                m2 = consts.tile([128, 1], F32, tag=f"m2{h}", name=f"m2{h}")
                nc.vector.tensor_mul(m2, mv[:, 0:1], mv[:, 0:1])
                nc.vector.tensor_add(m2, m2, mv[:, 1:2])
                nc.scalar.mul(out=sums[h][:, 0:1], in_=mv[:, 0:1], mul=n_loc)
                nc.scalar.mul(out=sums[h][:, 1:2], in_=m2, mul=n_loc)
                nc.gpsimd.dma_start(out=cc_in[h], in_=sums[h])
                nc.gpsimd.collective_compute(
                    "AllGather", ALU.bypass,
                    replica_groups=[list(range(N_CORES))],
                    ins=[cc_in[h][:, :]], outs=[cc_out[h][:, :, :]],
                )

            def post_ag(h):
                """Combine 8 cores' (S, SS); gh = gamma/std, dl = beta-gh*mean."""
                for r in range(N_CORES):
                    nc.gpsimd.dma_start(out=ag_t[h][:, r, :], in_=cc_out[h][r])
                a = ag_t[h]
                nc.vector.tensor_add(a[:, 0:4, :], a[:, 0:4, :], a[:, 4:8, :])
                nc.vector.tensor_add(a[:, 0:2, :], a[:, 0:2, :], a[:, 2:4, :])
                nc.vector.tensor_add(a[:, 0:1, :], a[:, 0:1, :], a[:, 1:2, :])
                gmean = consts.tile([128, 1], F32, tag=f"gmean{h}")
                var = consts.tile([128, 1], F32, tag=f"var{h}")
                tmp = consts.tile([128, 1], F32, tag=f"tmpe{h}")
                nc.scalar.mul(out=gmean, in_=a[:, 0, 0:1], mul=1.0 / N_PER_CH)
                nc.scalar.mul(out=var, in_=a[:, 0, 1:2], mul=1.0 / N_PER_CH)
                nc.vector.tensor_mul(tmp, gmean, gmean)
                nc.vector.tensor_sub(var, var, tmp)
                nc.scalar.activation(
                    out=var, in_=var, func=mybir.ActivationFunctionType.Sqrt,
                    bias=eps_t, scale=1.0,
                )
                nc.vector.reciprocal(out=var, in_=var)
                nc.vector.tensor_mul(gh_t[h], gbn_t[h][:, 0:1], var)
                nc.vector.tensor_mul(tmp, gmean, gh_t[h])
                nc.vector.tensor_sub(dl_t[h], gbn_t[h][:, 1:2], tmp)

            def emit_p2(h, b, xsrc, st1="pool"):
                """Pass-2: out = (y*gh + dl) + x; stage-2 + out DMA on DVE."""
                tmp = tmp16p.tile([128, TW], F16, tag="tmp")
                if st1 == "dve":
                    nc.vector.tensor_scalar(
                        out=tmp, in0=y16[h][:, b, :],
                        scalar1=gh_t[h], scalar2=dl_t[h],
                        op0=ALU.mult, op1=ALU.add,
                    )
                elif st1 == "act":
                    nc.scalar.activation(
                        out=tmp, in_=y16[h][:, b, :],
                        func=mybir.ActivationFunctionType.Identity,
                        bias=dl_t[h], scale=gh_t[h],
                    )
                else:
                    nc.gpsimd.tensor_scalar(
                        out=tmp, in0=y16[h][:, b, :],
                        scalar1=gh_t[h], scalar2=dl_t[h],
                        op0=ALU.mult, op1=ALU.add,
                    )
                nc.vector.tensor_add(tmp, tmp, xsrc[:, :TW])
                nc.sync.dma_start(
                    out=out_d[b, h * 128:(h + 1) * 128, :], in_=tmp,
                )

            # ---- pass 1 h0 (x-h1 prefetched into resident tiles) ----
            xt_h0 = []
            for b in range(2):
                xt = xp.tile([128, TW_PAD], F16, tag="xt")
                nc.sync.dma_start(out=xt, in_=x16_in[b, 0])
                xt_h0.append(xt)
            for b in range(B_LOC):
                emit_unit(0, b, xt_h0[b])
                if b + 2 < B_LOC:
                    xt = xp.tile([128, TW_PAD], F16, tag="xt")
                    nc.sync.dma_start(out=xt, in_=x16_in[b + 2, 0])
                    xt_h0.append(xt)
                nc.sync.dma_start(out=xres[b], in_=x16_in[b, 1])
            stats_epilogue(0)

            # pass-2 h0 x re-reads can start as soon as the xp pool frees
            xt2 = []
            for b in range(3):
                t = xp.tile([128, TW_PAD], F16, tag="xt")
                nc.sync.dma_start(out=t[:, :TW], in_=x16_in[b, 0, :, :TW])
                xt2.append(t)

            # ---- pass 1 h1 interleaved with pass 2 h0 ----
            for k in range(B_LOC):
                emit_unit(1, k, xres[k])
                if k == 3:
                    post_ag(0)
                if k >= 4:
                    for b in (2 * (k - 4), 2 * (k - 4) + 1):
                        emit_p2(0, b, xt2[b])
                        if b + 3 < B_LOC:
                            t = xp.tile([128, TW_PAD], F16, tag="xt")
                            nc.sync.dma_start(
                                out=t[:, :TW], in_=x16_in[b + 3, 0, :, :TW]
                            )
                            xt2.append(t)
            stats_epilogue(1)
            post_ag(1)

            # ---- pass 2 h1 (x resident in SBUF, write-only tail) ----
            ST1_H1 = {0: "dve", 1: "pool", 2: "dve", 3: "pool", 4: "dve",
                      5: "pool", 6: "dve", 7: "pool"}
            for b in range(B_LOC):
                emit_p2(1, b, xres[b], st1=ST1_H1[b])

    nc.finalize()
    return nc


def _prep_consts(A, A_group, conv_w, gamma, beta):
    A_sum = A.sum(axis=0)
    row_sum = np.clip(A_sum.sum(axis=-1, keepdims=True), 1e-6, None)
    A_g = (A_sum / row_sum)[None, :, :] + A_group          # (4,25,25)
    wblk = np.zeros((2, 128, 128), np.float16)
    for h in range(2):
        for gl in range(2):
            g = 2 * h + gl
            wblk[h, gl * 64:(gl + 1) * 64, gl * 64:(gl + 1) * 64] = \
                conv_w[g].T.astype(np.float16)
    eye = np.eye(5, dtype=np.float32)
    arhs = np.stack([np.kron(eye, A_g[g].T) for g in range(G)]).astype(np.float16)
    gbn = np.stack(
        [np.stack([gamma.reshape(2, 128)[h], beta.reshape(2, 128)[h]], axis=1)
         for h in range(2)]
    ).astype(np.float32)
    return wblk, np.ascontiguousarray(arhs), np.ascontiguousarray(gbn)


def _run(inputs, trace=False, **kw):
    if "nc" not in _cache:
        _cache["nc"] = _build()
    nc = _cache["nc"]
    x = np.asarray(inputs["x"], dtype=np.float32)
    wblk, arhs, gbn = _prep_consts(
        np.asarray(inputs["A"], np.float32),
        np.asarray(inputs["A_group"], np.float32),
        np.asarray(inputs["conv_w"], np.float32),
        np.asarray(inputs["gamma"], np.float32),
        np.asarray(inputs["beta"], np.float32),
    )
    xs = x.reshape(N_CORES, B_LOC, 2, 128, TW)
    x16 = np.zeros((N_CORES, B_LOC, 2, 128, TW_PAD), np.float16)
    x16[..., :TW] = xs.astype(np.float16)
    in_maps = [
        {"x16": np.ascontiguousarray(x16[i]), "wblk": wblk, "arhs": arhs, "gbn": gbn}
        for i in range(N_CORES)
    ]
    res = run_bass_kernel_spmd(nc, in_maps, list(range(N_CORES)), trace=trace, **kw)
    out = np.concatenate([res.results[i]["out"][None] for i in range(N_CORES)])
    return out.astype(np.float32).reshape(B, C, T, V), res


def kernel(**inputs) -> np.ndarray:
    out, _ = _run(inputs)
    return out
